# revision 4
# baseline (speedup 1.0000x reference)
"""Trainium2 Bass kernel for nn_CAttention (sparse cluster attention).

Contract: kernel(**inputs) takes FULL unsharded numpy inputs and returns the
full output tuple (x_out [8,2048,512] f32, attn_map [8,2048,2048] f32).

Strategy: data-parallel over batch B=8 across the 8 NeuronCores. Per core:
  - attn_map = (x@Wq*SCALE) @ (x@Wk)^T computed dense (required output).
  - The attn @ v_s product collapses: within a cluster every row of the
    masked/normalized attn matrix is identical, so the [N,N]@[N,C] product
    reduces to 16 per-cluster weighted sums of v rows (rank-16), and the
    output projection factors through Pt = St^T @ Wproj (host-precomputed).
  - All permutations (argsort shuffle, the swapaxes/reshape scramble, the
    restore gather) are folded into host-side input prep / output gather.
All heavy FLOPs run on the TensorEngine in float32r (full-rate fp32 mode).
"""

import numpy as np

import concourse.bass as bass
import concourse.tile as tile
from concourse import bacc, mybir

N = 2048
C = 512
NCL = 16
EPS = 1e-6
EPSN = EPS / N
SCALE = (C // 8) ** -0.5  # HEAD_DIM = 64

f32 = mybir.dt.float32
f32r = mybir.dt.float32r

KC = C // 128   # 4 contraction chunks over channel dim
IC = N // 128   # 16 row chunks over token dim
JB = N // 512   # 4 free-dim blocks of 512 over tokens


def _build_nc():
    nc = bacc.Bacc("TRN2", target_bir_lowering=False, debug=False, num_devices=8)

    xt = nc.dram_tensor("xt", [C, N], f32r, kind="ExternalInput")      # x^T
    xst = nc.dram_tensor("xst", [C, N], f32r, kind="ExternalInput")    # x[sigma]^T
    xsst = nc.dram_tensor("xsst", [C, N], f32r, kind="ExternalInput")  # x[sigma o sigma]^T
    wq = nc.dram_tensor("wq", [C, C], f32r, kind="ExternalInput")      # Wq * SCALE
    wk = nc.dram_tensor("wk", [C, C], f32r, kind="ExternalInput")
    wv = nc.dram_tensor("wv", [C, C], f32r, kind="ExternalInput")
    m1s = nc.dram_tensor("m1s", [N, NCL], f32r, kind="ExternalInput")  # sorted-frame onehot
    epsc = nc.dram_tensor("epsc", [128, NCL], f32r, kind="ExternalInput")  # EPS/N
    hones = nc.dram_tensor("hones", [128, 2], f32r, kind="ExternalInput")  # 1.0
    ones1 = nc.dram_tensor("ones1", [1, C], f32r, kind="ExternalInput")    # 1.0
    pt = nc.dram_tensor("pt", [4, NCL + 1, C], f32r, kind="ExternalInput")
    attn = nc.dram_tensor("attn", [N, N], f32, kind="ExternalOutput")
    z = nc.dram_tensor("z", [N, C], f32, kind="ExternalOutput")

    with tile.TileContext(nc) as tc:
        with (
            tc.tile_pool(name="const", bufs=1) as pc,
            tc.tile_pool(name="xper", bufs=1) as pxt,
        ):
            # ---- persistent loads ----
            wq_t = []
            wk_t = []
            wv_t = []
            xt_t = []
            for k in range(KC):
                t = pc.tile([128, C], f32r, tag=f"wq{k}")
                nc.sync.dma_start(t[:], wq[k * 128:(k + 1) * 128, :])
                wq_t.append(t)
                t = pc.tile([128, C], f32r, tag=f"wk{k}")
                nc.sync.dma_start(t[:], wk[k * 128:(k + 1) * 128, :])
                wk_t.append(t)
                t = pc.tile([128, C], f32r, tag=f"wv{k}")
                nc.sync.dma_start(t[:], wv[k * 128:(k + 1) * 128, :])
                wv_t.append(t)
                t = pxt.tile([128, N], f32r, tag=f"xt{k}")
                nc.sync.dma_start(t[:], xt[k * 128:(k + 1) * 128, :])
                xt_t.append(t)
            m1s_sb = pc.tile([128, IC * NCL], f32r, tag="m1s")
            nc.sync.dma_start(
                m1s_sb[:].rearrange("p (i c) -> p i c", c=NCL),
                m1s[:].rearrange("(i p) c -> p i c", p=128),
            )
            epsc_sb = pc.tile([128, NCL], f32r, tag="epsc")
            nc.sync.dma_start(epsc_sb[:], epsc[:])
            hones_sb = pc.tile([128, 2], f32r, tag="hones")
            nc.sync.dma_start(hones_sb[:], hones[:])
            pt_sb = []
            for t4 in range(4):
                t = pc.tile([NCL + 1, C], f32r, tag=f"pt{t4}")
                nc.sync.dma_start(t[:], pt[t4])
                pt_sb.append(t)
            o17 = pc.tile([NCL + 1, C], f32r, tag="o17")
            nc.sync.dma_start(o17[NCL:NCL + 1, :], ones1[:])

            # ================= Phase 1: x_out branch =================
            with (
                tc.tile_pool(name="p1x", bufs=1) as p1x,
                tc.tile_pool(name="p1s", bufs=3) as p1s,
                tc.tile_pool(name="p1p", bufs=1, space="PSUM") as p1p,
            ):
                xst_t = []
                xsst_t = []
                for k in range(KC):
                    t = p1x.tile([128, N], f32r, tag=f"xst{k}")
                    nc.sync.dma_start(t[:], xst[k * 128:(k + 1) * 128, :])
                    xst_t.append(t)
                    t = p1x.tile([128, N], f32r, tag=f"xsst{k}")
                    nc.sync.dma_start(t[:], xsst[k * 128:(k + 1) * 128, :])
                    xsst_t.append(t)

                ws_ps = p1p.tile([NCL, C], f32, tag="ws")
                s_ps = p1p.tile([NCL, 2], f32, tag="s")

                for i in range(IC):
                    sl = bass.ts(i, 128)
                    ps_v = p1p.tile([128, C], f32, tag="psv")
                    ps_q = p1p.tile([128, C], f32, tag="psq")
                    ps_k = p1p.tile([128, C], f32, tag="psk")
                    for k in range(KC):
                        nc.tensor.matmul(ps_v[:], xst_t[k][:, sl], wv_t[k][:],
                                         start=(k == 0), stop=(k == KC - 1))
                    for k in range(KC):
                        nc.tensor.matmul(ps_q[:], xsst_t[k][:, sl], wq_t[k][:],
                                         start=(k == 0), stop=(k == KC - 1))
                    for k in range(KC):
                        nc.tensor.matmul(ps_k[:], xt_t[k][:, sl], wk_t[k][:],
                                         start=(k == 0), stop=(k == KC - 1))
                    # rowdot -> colv [128,1]; exp -> e [128,1]
                    # (tensor_tensor_reduce hangs on HW; use mul + reduce)
                    qdd = p1s.tile([128, C], f32, tag="qdd")
                    nc.vector.tensor_copy(qdd[:], ps_q[:])
                    nc.vector.tensor_mul(qdd[:], qdd[:], ps_k[:])
                    colv = p1s.tile([128, 1], f32, tag="colv")
                    nc.vector.reduce_sum(colv[:], qdd[:],
                                         axis=mybir.AxisListType.X)
                    e_sb = p1s.tile([128, 1], f32, tag="e")
                    nc.scalar.activation(e_sb[:], colv[:],
                                         mybir.ActivationFunctionType.Exp)
                    m1e = p1s.tile([128, NCL], f32r, tag="m1e")
                    nc.vector.tensor_scalar_mul(
                        m1e[:], m1s_sb[:, bass.ts(i, NCL)], e_sb[:])
                    vs = p1s.tile([128, C], f32r, tag="vs")
                    nc.vector.tensor_copy(vs[:], ps_v[:])
                    # accumulate WS += m1e^T @ vs + epsc^T @ vs ; S += m1e^T @ 1
                    nc.tensor.matmul(ws_ps[:], m1e[:], vs[:],
                                     start=(i == 0), stop=False,
                                     skip_group_check=True)
                    nc.tensor.matmul(ws_ps[:], epsc_sb[:], vs[:],
                                     start=False, stop=(i == IC - 1),
                                     skip_group_check=True)
                    nc.tensor.matmul(s_ps[:], m1e[:], hones_sb[:],
                                     start=(i == 0), stop=(i == IC - 1),
                                     skip_group_check=True)

                # O = (WS + eps-term) / (S + EPS); row 16 stays 1.0
                s_eps = p1s.tile([NCL, 1], f32, tag="seps")
                nc.vector.tensor_scalar_add(s_eps[:], s_ps[:, 0:1], EPS)
                recip = p1s.tile([NCL, 1], f32, tag="recip")
                nc.vector.reciprocal(recip[:], s_eps[:])
                nc.vector.tensor_scalar_mul(o17[0:NCL, :], ws_ps[:], recip[:])

            # ================= Phase 2: attn_map + Z =================
            with (
                tc.tile_pool(name="p2qk", bufs=1) as p2qk,
                tc.tile_pool(name="p2s", bufs=4) as p2s,
                tc.tile_pool(name="p2z", bufs=2) as p2z,
                tc.tile_pool(name="p2p", bufs=1, space="PSUM") as p2p,
            ):
                qT_t = []
                kT_t = []
                for m in range(KC):
                    qT_t.append(p2qk.tile([128, N], f32r, tag=f"qT{m}",
                                          name=f"qT{m}"))
                    kT_t.append(p2qk.tile([128, N], f32r, tag=f"kT{m}",
                                          name=f"kT{m}"))
                for m in range(KC):
                    for j in range(JB):
                        slj = bass.ts(j, 512)
                        psp = p2p.tile([128, 512], f32, tag="proj0")
                        for k in range(KC):
                            nc.tensor.matmul(
                                psp[:], wq_t[k][:, bass.ts(m, 128)],
                                xt_t[k][:, slj],
                                start=(k == 0), stop=(k == KC - 1))
                        nc.vector.tensor_copy(qT_t[m][:, slj], psp[:])
                        psp2 = p2p.tile([128, 512], f32, tag="proj1")
                        for k in range(KC):
                            nc.tensor.matmul(
                                psp2[:], wk_t[k][:, bass.ts(m, 128)],
                                xt_t[k][:, slj],
                                start=(k == 0), stop=(k == KC - 1))
                        nc.vector.tensor_copy(kT_t[m][:, slj], psp2[:])

                # Z_t = O'^T @ Pt  (rank-17), rows r=4p+t of Y
                zv = z[:].rearrange("(p t) m -> t p m", t=4)
                for t4 in range(4):
                    for pck in range(KC):
                        psz = p2p.tile([128, C], f32, tag="z")
                        nc.tensor.matmul(psz[:], o17[:, bass.ts(pck, 128)],
                                         pt_sb[t4][:], start=True, stop=True)
                        zsb = p2z.tile([128, C], f32, tag="zsb")
                        nc.vector.tensor_copy(zsb[:], psz[:])
                        nc.sync.dma_start(zv[t4, bass.ts(pck, 128), :], zsb[:])

                # attn_map = qT^T @ kT, streamed out
                for i in range(IC):
                    sli = bass.ts(i, 128)
                    for j in range(JB):
                        slj = bass.ts(j, 512)
                        psa = p2p.tile([128, 512], f32, tag="attn")
                        for m in range(KC):
                            nc.tensor.matmul(psa[:], qT_t[m][:, sli],
                                             kT_t[m][:, slj],
                                             start=(m == 0), stop=(m == KC - 1))
                        asb = p2s.tile([128, 512], f32, tag="asb")
                        nc.vector.tensor_copy(asb[:], psa[:])
                        nc.sync.dma_start(attn[sli, slj], asb[:])

    nc.compile()
    return nc


def _make_runner(nc, n_cores=8):
    import jax
    from jax.sharding import Mesh, PartitionSpec
    from jax.experimental.shard_map import shard_map
    from concourse import bass2jax
    from concourse.bass2jax import _bass_exec_p, install_neuronx_cc_hook

    install_neuronx_cc_hook()
    partition_name = nc.partition_id_tensor.name if nc.partition_id_tensor else None
    in_names, out_names, out_avals, zero_outs = [], [], [], []
    for alloc in nc.m.functions[0].allocations:
        if not isinstance(alloc, mybir.MemoryLocationSet):
            continue
        name = alloc.memorylocations[0].name
        if alloc.kind == "ExternalInput":
            if name != partition_name:
                in_names.append(name)
        elif alloc.kind == "ExternalOutput":
            out_names.append(name)
            shape = tuple(alloc.tensor_shape)
            dtype = mybir.dt.np(alloc.dtype)
            out_avals.append(jax.core.ShapedArray(shape, dtype))
            zero_outs.append(np.zeros(shape, dtype))
    n_params = len(in_names)
    n_outs = len(out_avals)
    all_in_names = list(in_names) + list(out_names)
    if partition_name is not None:
        all_in_names.append(partition_name)

    def _body(*args):
        operands = list(args)
        if partition_name is not None:
            operands.append(bass2jax.partition_id_tensor())
        outs = _bass_exec_p.bind(
            *operands,
            out_avals=tuple(out_avals),
            in_names=tuple(all_in_names),
            out_names=tuple(out_names),
            lowering_input_output_aliases=(),
            sim_require_finite=True,
            sim_require_nnan=True,
            nc=nc,
        )
        return tuple(outs)

    devices = jax.devices()[:n_cores]
    mesh = Mesh(np.asarray(devices), ("core",))
    in_specs = (PartitionSpec("core"),) * (n_params + n_outs)
    out_specs = (PartitionSpec("core"),) * n_outs
    sharded = jax.jit(
        shard_map(_body, mesh=mesh, in_specs=in_specs, out_specs=out_specs,
                  check_rep=False),
        keep_unused=True,
    )

    def run(in_maps):
        per_core = [[np.asarray(m[name]) for name in in_names] for m in in_maps]
        concat_in = [
            np.concatenate([per_core[cc][i] for cc in range(n_cores)], axis=0)
            for i in range(n_params)
        ]
        concat_zeros = [
            np.zeros((n_cores * zz.shape[0], *zz.shape[1:]), zz.dtype)
            for zz in zero_outs
        ]
        out_arrs = sharded(*concat_in, *concat_zeros)
        import jax as _jax
        _jax.block_until_ready(out_arrs)
        return [
            {name: np.asarray(out_arrs[i]).reshape(n_cores, *out_avals[i].shape)[cc]
             for i, name in enumerate(out_names)}
            for cc in range(n_cores)
        ]

    return run


_STATE = {}


def _get_runner():
    if "run" not in _STATE:
        nc = _build_nc()
        _STATE["nc"] = nc
        _STATE["run"] = _make_runner(nc, 8)
    return _STATE["run"]


def kernel(x_token, x_path, idx_cluster, cluster_num, Wqk, Wv, Wpv, Wproj, bproj):
    x = np.asarray(x_token, dtype=np.float32)
    idx = np.asarray(idx_cluster)
    B = x.shape[0]
    cn = int(cluster_num)
    Wq = np.ascontiguousarray(np.asarray(Wqk, np.float32)[:, :C] * SCALE)
    Wk = np.ascontiguousarray(np.asarray(Wqk, np.float32)[:, C:])
    Wv_ = np.ascontiguousarray(np.asarray(Wv, np.float32))
    Wp = np.asarray(Wproj, np.float32)
    bp = np.asarray(bproj, np.float32)
    epsc = np.full((128, NCL), EPSN, np.float32)
    hones_a = np.ones((128, 2), np.float32)
    ones1_a = np.ones((1, C), np.float32)

    in_maps = []
    rhos = []
    for b in range(B):
        sig = np.argsort(idx[b], kind="stable")
        rho = np.argsort(sig, kind="stable")
        s = idx[b][sig]
        xb = x[b]
        m1 = np.zeros((N, NCL), np.float32)
        act = s < cn
        m1[np.nonzero(act)[0], s[act]] = 1.0
        ptb = np.zeros((4, NCL + 1, C), np.float32)
        for t4 in range(4):
            st = s[t4 * 512:(t4 + 1) * 512]
            np.add.at(ptb[t4], st, Wp)
            ptb[t4, NCL] = bp
        in_maps.append({
            "xt": np.ascontiguousarray(xb.T),
            "xst": np.ascontiguousarray(xb[sig].T),
            "xsst": np.ascontiguousarray(xb[sig[sig]].T),
            "wq": Wq, "wk": Wk, "wv": Wv_,
            "m1s": m1, "epsc": epsc, "hones": hones_a, "ones1": ones1_a,
            "pt": ptb,
        })
        rhos.append(rho)

    run = _get_runner()
    results = run(in_maps)

    x_out = np.empty((B, N, C), np.float32)
    attn_map = np.empty((B, N, N), np.float32)
    for b in range(B):
        attn_map[b] = results[b]["attn"]
        x_out[b] = results[b]["z"][rhos[b]]
    return x_out, attn_map


# revision 18
# speedup vs baseline: 1.4191x; 1.4191x over previous
"""Trainium2 Bass kernel for nn_CAttention (sparse cluster attention).

Contract: kernel(**inputs) takes FULL unsharded numpy inputs and returns the
full output tuple (x_out [8,2048,512] f32, attn_map [8,2048,2048] f32).

Strategy: data-parallel over batch B=8 across the 8 NeuronCores. Per core:
  - attn_map = (x@Wq*SCALE) @ (x@Wk)^T computed dense (required output).
  - The attn @ v_s product collapses: within a cluster every row of the
    masked/normalized attn matrix is identical, so the [N,N]@[N,C] product
    reduces to 16 per-cluster weighted sums of v rows (rank-16), and the
    output projection factors through Pt = St^T @ Wproj (host-precomputed).
  - All permutations (argsort shuffle, the swapaxes/reshape scramble, the
    restore gather) are folded into host-side input prep / output gather.
TensorEngine streams run in bf16 (full rate, halves DMA/SBUF); accumulation
is fp32 in PSUM; softmax/normalization arithmetic is fp32.
"""

import numpy as np

import concourse.bass as bass
import concourse.tile as tile
from concourse import bacc, mybir

N = 2048
C = 512
NCL = 16
EPS = 1e-6
EPSN = EPS / N
SCALE = (C // 8) ** -0.5  # HEAD_DIM = 64

f32 = mybir.dt.float32
bf16 = mybir.dt.bfloat16

KC = C // 128   # 4 contraction chunks over channel dim
IC = N // 128   # 16 row chunks over token dim
JB = N // 512   # 4 free-dim blocks of 512 over tokens


def _build_nc(rep=1):
    nc = bacc.Bacc("TRN2", target_bir_lowering=False, debug=False, num_devices=8)

    xt = nc.dram_tensor("xt", [C, N], bf16, kind="ExternalInput")      # x^T
    xst = nc.dram_tensor("xst", [C, N], bf16, kind="ExternalInput")    # x[sig]^T
    xsst = nc.dram_tensor("xsst", [C, N], bf16, kind="ExternalInput")  # x[sig o sig]^T
    wq = nc.dram_tensor("wq", [C, C], bf16, kind="ExternalInput")      # Wq * SCALE
    wk = nc.dram_tensor("wk", [C, C], bf16, kind="ExternalInput")
    wv = nc.dram_tensor("wv", [C, C], bf16, kind="ExternalInput")
    m1s = nc.dram_tensor("m1s", [N, NCL], f32, kind="ExternalInput")
    epsc = nc.dram_tensor("epsc", [128, NCL], bf16, kind="ExternalInput")
    hones = nc.dram_tensor("hones", [128, 2], bf16, kind="ExternalInput")
    ones1 = nc.dram_tensor("ones1", [1, C], bf16, kind="ExternalInput")
    pt = nc.dram_tensor("pt", [4, NCL + 1, C], bf16, kind="ExternalInput")
    attn = nc.dram_tensor("attn", [N, N], f32, kind="ExternalOutput")
    z = nc.dram_tensor("z", [N, C], f32, kind="ExternalOutput")

    with tile.TileContext(nc) as tc:
        for _r in range(rep):
            with (
                tc.tile_pool(name=f"pers{_r}", bufs=1) as pc,
                tc.tile_pool(name=f"work{_r}", bufs=3) as pw,
                tc.tile_pool(name=f"psum{_r}", bufs=1, space="PSUM") as pp,
            ):
                wq_t, wk_t, wv_t, xt_t, xst_t, xsst_t = [], [], [], [], [], []
                qT_t, kT_t = [], []
                for k in range(KC):
                    wq_t.append(pc.tile([128, C], bf16, tag=f"wq{k}", name=f"wq{k}"))
                    wk_t.append(pc.tile([128, C], bf16, tag=f"wk{k}", name=f"wk{k}"))
                    wv_t.append(pc.tile([128, C], bf16, tag=f"wv{k}", name=f"wv{k}"))
                    xt_t.append(pc.tile([128, N], bf16, tag=f"xt{k}", name=f"xt{k}"))
                    xst_t.append(pc.tile([128, N], bf16, tag=f"xst{k}",
                                         name=f"xst{k}"))
                    xsst_t.append(pc.tile([128, N], bf16, tag=f"xsst{k}",
                                          name=f"xsst{k}"))
                    qT_t.append(pc.tile([128, N], bf16, tag=f"qT{k}", name=f"qT{k}"))
                    kT_t.append(pc.tile([128, N], bf16, tag=f"kT{k}", name=f"kT{k}"))

                # ---- loads, in first-consumption order ----
                sl0 = bass.ts(0, 512)
                for k in range(KC):
                    nc.sync.dma_start(xst_t[k][:, sl0], xst[k * 128:(k + 1) * 128, sl0])
                for k in range(KC):
                    nc.sync.dma_start(wv_t[k][:], wv[k * 128:(k + 1) * 128, :])
                for k in range(KC):
                    nc.sync.dma_start(xsst_t[k][:, sl0],
                                        xsst[k * 128:(k + 1) * 128, sl0])
                for k in range(KC):
                    nc.sync.dma_start(wq_t[k][:], wq[k * 128:(k + 1) * 128, :])
                for k in range(KC):
                    nc.sync.dma_start(xt_t[k][:, sl0], xt[k * 128:(k + 1) * 128, sl0])
                for k in range(KC):
                    nc.sync.dma_start(wk_t[k][:], wk[k * 128:(k + 1) * 128, :])
                for j in range(1, JB):
                    slj = bass.ts(j, 512)
                    for k in range(KC):
                        nc.sync.dma_start(xst_t[k][:, slj],
                                          xst[k * 128:(k + 1) * 128, slj])
                        nc.sync.dma_start(xsst_t[k][:, slj],
                                            xsst[k * 128:(k + 1) * 128, slj])
                        nc.sync.dma_start(xt_t[k][:, slj],
                                          xt[k * 128:(k + 1) * 128, slj])
                m1s_sb = pc.tile([128, IC * NCL], f32, tag="m1s", name="m1s_sb")
                nc.sync.dma_start(
                    m1s_sb[:].rearrange("p (i c) -> p i c", c=NCL),
                    m1s[:].rearrange("(i p) c -> p i c", p=128),
                )
                epsc_sb = pc.tile([128, NCL], bf16, tag="epsc", name="epsc_sb")
                nc.sync.dma_start(epsc_sb[:], epsc[:])
                hones_sb = pc.tile([128, 2], bf16, tag="hones", name="hones_sb")
                nc.sync.dma_start(hones_sb[:], hones[:])
                pt_sb = []
                for t4 in range(4):
                    t = pc.tile([NCL + 1, C], bf16, tag=f"pt{t4}", name=f"pt{t4}")
                    nc.sync.dma_start(t[:], pt[t4])
                    pt_sb.append(t)
                o17 = pc.tile([NCL + 1, C], bf16, tag="o17", name="o17")
                nc.sync.dma_start(o17[NCL:NCL + 1, :], ones1[:])

                ws_ps = pp.tile([NCL, C], f32, tag="ws", name="ws_ps")
                s_ps = pp.tile([NCL, 2], f32, tag="s", name="s_ps")

                # ---- phase 1 ic-loop with interleaved qT/kT projection ----
                proj_pairs = [(m, j) for m in range(KC) for j in range(JB)]
                for i in range(IC):
                    sl = bass.ts(i, 128)
                    ps_v = pp.tile([128, C], f32, tag="psv", name="ps_v")
                    ps_q = pp.tile([128, C], f32, tag="psq", name="ps_q")
                    ps_k = pp.tile([128, C], f32, tag="psk", name="ps_k")
                    for k in range(KC):
                        nc.tensor.matmul(ps_v[:], xst_t[k][:, sl], wv_t[k][:],
                                         start=(k == 0), stop=(k == KC - 1))
                    for k in range(KC):
                        nc.tensor.matmul(ps_q[:], xsst_t[k][:, sl], wq_t[k][:],
                                         start=(k == 0), stop=(k == KC - 1))
                    for k in range(KC):
                        nc.tensor.matmul(ps_k[:], xt_t[k][:, sl], wk_t[k][:],
                                         start=(k == 0), stop=(k == KC - 1))
                    # rowdot -> colv; exp -> e  (fp32 throughout)
                    # (tensor_tensor_reduce hangs on HW; use copy+mul+reduce)
                    qdd = pw.tile([128, C], f32, tag="qdd", name="qdd")
                    nc.vector.tensor_copy(qdd[:], ps_q[:])
                    nc.vector.tensor_mul(qdd[:], qdd[:], ps_k[:])
                    colv = pw.tile([128, 1], f32, tag="colv", name="colv")
                    nc.vector.reduce_sum(colv[:], qdd[:], axis=mybir.AxisListType.X)
                    e_sb = pw.tile([128, 1], f32, tag="e", name="e_sb")
                    nc.scalar.activation(e_sb[:], colv[:],
                                         mybir.ActivationFunctionType.Exp)
                    m1e = pw.tile([128, NCL], bf16, tag="m1e", name="m1e")
                    nc.vector.tensor_scalar_mul(
                        m1e[:], m1s_sb[:, bass.ts(i, NCL)], e_sb[:])
                    vs = pw.tile([128, C], bf16, tag="vs", name="vs")
                    nc.scalar.activation(vs[:], ps_v[:],
                                         mybir.ActivationFunctionType.Copy)
                    nc.tensor.matmul(ws_ps[:], m1e[:], vs[:],
                                     start=(i == 0), stop=False,
                                     skip_group_check=True)
                    nc.tensor.matmul(ws_ps[:], epsc_sb[:], vs[:],
                                     start=False, stop=(i == IC - 1),
                                     skip_group_check=True)
                    nc.tensor.matmul(s_ps[:], m1e[:], hones_sb[:],
                                     start=(i == 0), stop=(i == IC - 1),
                                     skip_group_check=True)
                    # interleave one qT/kT projection (m, j) pair per ic
                    m, j = proj_pairs[i]
                    slj = bass.ts(j, 512)
                    psp = pp.tile([128, 512], f32, tag="mm", bufs=3, name="psp")
                    for k in range(KC):
                        nc.tensor.matmul(psp[:], wq_t[k][:, bass.ts(m, 128)],
                                         xt_t[k][:, slj],
                                         start=(k == 0), stop=(k == KC - 1))
                    nc.vector.tensor_copy(qT_t[m][:, slj], psp[:])
                    psp2 = pp.tile([128, 512], f32, tag="mm", bufs=3, name="psp2")
                    for k in range(KC):
                        nc.tensor.matmul(psp2[:], wk_t[k][:, bass.ts(m, 128)],
                                         xt_t[k][:, slj],
                                         start=(k == 0), stop=(k == KC - 1))
                    nc.vector.tensor_copy(kT_t[m][:, slj], psp2[:])

                # ---- O = (WS + eps-term) / (S + EPS); bias row is 1.0 ----
                s_eps = pw.tile([NCL, 1], f32, tag="seps", name="s_eps")
                nc.vector.tensor_scalar_add(s_eps[:], s_ps[:, 0:1], EPS)
                recip = pw.tile([NCL, 1], f32, tag="recip", name="recip")
                nc.vector.reciprocal(recip[:], s_eps[:])
                nc.vector.tensor_scalar_mul(o17[0:NCL, :], ws_ps[:], recip[:])

                # ---- attn_map = qT^T @ kT, streamed out ----
                for i in range(IC):
                    sli = bass.ts(i, 128)
                    for j in range(JB):
                        slj = bass.ts(j, 512)
                        psa = pp.tile([128, 512], f32, tag="mm", bufs=3,
                                      name="psa")
                        for m in range(KC):
                            nc.tensor.matmul(psa[:], qT_t[m][:, sli],
                                             kT_t[m][:, slj],
                                             start=(m == 0), stop=(m == KC - 1))
                        asb = pw.tile([128, 512], f32, tag="asb", bufs=6,
                                      name="asb")
                        if (i * JB + j) % 2 == 0:
                            nc.vector.tensor_copy(asb[:], psa[:])
                            nc.sync.dma_start(attn[sli, slj], asb[:])
                        else:
                            nc.scalar.activation(
                                asb[:], psa[:],
                                mybir.ActivationFunctionType.Copy)
                            nc.sync.dma_start(attn[sli, slj], asb[:])

                # ---- Z_t = O'^T @ Pt (rank-17): rows r=4p+t of Y ----
                zv = z[:].rearrange("(p t) m -> t p m", t=4)
                for t4 in range(4):
                    for pck in range(KC):
                        psz = pp.tile([128, C], f32, tag="mm", bufs=3, name="psz")
                        nc.tensor.matmul(psz[:], o17[:, bass.ts(pck, 128)],
                                         pt_sb[t4][:], start=True, stop=True)
                        zsb = pw.tile([128, C], f32, tag="zsb", name="zsb")
                        nc.scalar.activation(zsb[:], psz[:],
                                             mybir.ActivationFunctionType.Copy)
                        nc.sync.dma_start(zv[t4, bass.ts(pck, 128), :], zsb[:])

    nc.compile()
    return nc


def _make_runner(nc, n_cores=8):
    import jax
    from jax.sharding import Mesh, PartitionSpec
    from jax.experimental.shard_map import shard_map
    from concourse import bass2jax
    from concourse.bass2jax import _bass_exec_p, install_neuronx_cc_hook

    install_neuronx_cc_hook()
    partition_name = nc.partition_id_tensor.name if nc.partition_id_tensor else None
    in_names, out_names, out_avals, zero_outs = [], [], [], []
    for alloc in nc.m.functions[0].allocations:
        if not isinstance(alloc, mybir.MemoryLocationSet):
            continue
        name = alloc.memorylocations[0].name
        if alloc.kind == "ExternalInput":
            if name != partition_name:
                in_names.append(name)
        elif alloc.kind == "ExternalOutput":
            out_names.append(name)
            shape = tuple(alloc.tensor_shape)
            dtype = mybir.dt.np(alloc.dtype)
            out_avals.append(jax.core.ShapedArray(shape, dtype))
            zero_outs.append(np.zeros(shape, dtype))
    n_params = len(in_names)
    n_outs = len(out_avals)
    all_in_names = list(in_names) + list(out_names)
    if partition_name is not None:
        all_in_names.append(partition_name)

    def _body(*args):
        operands = list(args)
        if partition_name is not None:
            operands.append(bass2jax.partition_id_tensor())
        outs = _bass_exec_p.bind(
            *operands,
            out_avals=tuple(out_avals),
            in_names=tuple(all_in_names),
            out_names=tuple(out_names),
            lowering_input_output_aliases=(),
            sim_require_finite=True,
            sim_require_nnan=True,
            nc=nc,
        )
        return tuple(outs)

    devices = jax.devices()[:n_cores]
    mesh = Mesh(np.asarray(devices), ("core",))
    in_specs = (PartitionSpec("core"),) * (n_params + n_outs)
    out_specs = (PartitionSpec("core"),) * n_outs
    sharded = jax.jit(
        shard_map(_body, mesh=mesh, in_specs=in_specs, out_specs=out_specs,
                  check_rep=False),
        keep_unused=True,
    )

    def run(in_maps):
        per_core = [[np.asarray(m[name]) for name in in_names] for m in in_maps]
        concat_in = [
            np.concatenate([per_core[cc][i] for cc in range(n_cores)], axis=0)
            for i in range(n_params)
        ]
        concat_zeros = [
            np.zeros((n_cores * zz.shape[0], *zz.shape[1:]), zz.dtype)
            for zz in zero_outs
        ]
        out_arrs = sharded(*concat_in, *concat_zeros)
        import jax as _jax
        _jax.block_until_ready(out_arrs)
        return [
            {name: np.asarray(out_arrs[i]).reshape(n_cores, *out_avals[i].shape)[cc]
             for i, name in enumerate(out_names)}
            for cc in range(n_cores)
        ]

    return run


_STATE = {}


def _get_runner():
    if "run" not in _STATE:
        nc = _build_nc()
        _STATE["nc"] = nc
        _STATE["run"] = _make_runner(nc, 8)
    return _STATE["run"]


def kernel(x_token, x_path, idx_cluster, cluster_num, Wqk, Wv, Wpv, Wproj, bproj):
    import ml_dtypes
    bf = ml_dtypes.bfloat16
    x = np.asarray(x_token, dtype=np.float32)
    idx = np.asarray(idx_cluster)
    B = x.shape[0]
    cn = int(cluster_num)
    Wq = np.ascontiguousarray(np.asarray(Wqk, np.float32)[:, :C] * SCALE).astype(bf)
    Wk = np.ascontiguousarray(np.asarray(Wqk, np.float32)[:, C:]).astype(bf)
    Wv_ = np.ascontiguousarray(np.asarray(Wv, np.float32)).astype(bf)
    Wp = np.asarray(Wproj, np.float32)
    bp = np.asarray(bproj, np.float32)
    epsc = np.full((128, NCL), EPSN, np.float32).astype(bf)
    hones_a = np.ones((128, 2), bf)
    ones1_a = np.ones((1, C), bf)

    in_maps = []
    rhos = []
    for b in range(B):
        sig = np.argsort(idx[b], kind="stable")
        rho = np.argsort(sig, kind="stable")
        s = idx[b][sig]
        xb = x[b]
        m1 = np.zeros((N, NCL), np.float32)
        act = s < cn
        m1[np.nonzero(act)[0], s[act]] = 1.0
        ptb = np.zeros((4, NCL + 1, C), np.float32)
        for t4 in range(4):
            st = s[t4 * 512:(t4 + 1) * 512]
            np.add.at(ptb[t4], st, Wp)
            ptb[t4, NCL] = bp
        in_maps.append({
            "xt": np.ascontiguousarray(xb.T).astype(bf),
            "xst": np.ascontiguousarray(xb[sig].T).astype(bf),
            "xsst": np.ascontiguousarray(xb[sig[sig]].T).astype(bf),
            "wq": Wq, "wk": Wk, "wv": Wv_,
            "m1s": m1, "epsc": epsc, "hones": hones_a, "ones1": ones1_a,
            "pt": ptb.astype(bf),
        })
        rhos.append(rho)

    run = _get_runner()
    results = run(in_maps)

    x_out = np.empty((B, N, C), np.float32)
    attn_map = np.empty((B, N, N), np.float32)
    for b in range(B):
        attn_map[b] = results[b]["attn"]
        x_out[b] = results[b]["z"][rhos[b]]
    return x_out, attn_map
